# revision 3
# baseline (speedup 1.0000x reference)
"""Trainium2 Bass kernel for 16-head self-attention (b=2, n=2048, dm=1024, dh=64).

Sharding: each of 8 cores owns (batch g = c//4, sequence block r = c%4).
A core computes K,V for its batch's FULL sequence (replicated across the 4
cores of that batch -- avoids cross-core collectives entirely), attention for
all 16 heads restricted to its 512 query rows, and the output projection for
those rows.  Per-core outputs are disjoint [512, 1024] slices of the final
[2, 2048, 1024]; the host concatenates.

Key layout trick: the host passes x^T (dm-major) ROTATED by the core's row
offset, so every core's query slice is columns 0:512 of its own x^T -- the
SPMD program is identical across cores.  Attention is permutation-invariant
over keys, so the rotation does not change the result.

All matmuls use float32r (4-xbus fp32 streaming: 1 cycle/row when the moving
free dim is >= 256, vs 4 for plain fp32; ~tf32 effective precision, measured
~2e-4 rel err end-to-end).  No on-chip transposes are needed anywhere:
  Q^T[i,q]  = (Wq  as lhsT)  @ (x^T as rhs)
  K^T[i,k]  = (Wk  as lhsT)  @ (x^T as rhs)
  V [k,i]   = (x^T as lhsT)  @ (Wv  as rhs)
  S^T[k,q]  = (K^T as lhsT)  @ (Q^T as rhs)          (per head, dh=64)
  O'' [d,q] = ([V|1] as lhsT) @ (exp(S^T) as rhs)    (row 64 = softmax denom)
  out[q,d]  = (O^T as lhsT)  @ (Wo  as rhs) + bo
Softmax denominators ride along as a 65th lhsT column of ones; the 1/denom
broadcast across partitions is a K=1 outer-product matmul with a ones vector
(bounced through SBUF: DVE may read only one PSUM operand per instruction).
"""

import sys

for _p in ("/opt/trn_rl_repo", "/root/.axon_site/_ro/trn_rl_repo"):
    if _p not in sys.path:
        sys.path.append(_p)

import numpy as np

B = 2
N = 2048
DM = 1024
H = 16
DH = 64
INNER = H * DH  # 1024
NCORES = 8
QR = 512  # query rows per core
SCALE = DH ** -0.5

_cached = {}


def _build():
    import contextlib
    import concourse.bacc as bacc
    import concourse.tile as tile
    import concourse.mybir as mybir

    f32 = mybir.dt.float32
    f32r = mybir.dt.float32r
    Exp = mybir.ActivationFunctionType.Exp

    nc = bacc.Bacc("TRN2", target_bir_lowering=False, debug=False,
                   enable_asserts=False)

    xT_d = nc.dram_tensor("xT", [DM, N], f32r, kind="ExternalInput").ap()
    Wq_d = nc.dram_tensor("Wq", [DM, INNER], f32r, kind="ExternalInput").ap()
    Wkv_d = nc.dram_tensor("Wkv", [DM, 2 * INNER], f32r, kind="ExternalInput").ap()
    Wo_d = nc.dram_tensor("Wo", [INNER, DM], f32r, kind="ExternalInput").ap()
    bo_d = nc.dram_tensor("bo", [DM], f32, kind="ExternalInput").ap()
    out_d = nc.dram_tensor("out", [QR, DM], f32, kind="ExternalOutput").ap()

    A = DM // 128      # 8 dm blocks
    IB = INNER // 128  # 8 inner blocks
    KB = N // 128      # 16 key blocks
    KC = N // 512      # 4 key chunks
    QB = QR // 128     # 4 query blocks

    with tile.TileContext(nc) as tc, \
         nc.allow_low_precision(reason="fp32r matmul pipeline, validated e2e"), \
         contextlib.ExitStack() as ctx:
            persist = ctx.enter_context(tc.tile_pool(name="persist", bufs=1))
            QT_sb = persist.tile([128, IB, QR], f32r)   # Q^T  [inner, q]
            OT_sb = persist.tile([128, IB, QR], f32r)   # O^T  [inner, q]
            bo_sb = persist.tile([128, DM], f32)
            onef = persist.tile([128, 1], f32)
            ones_sb = persist.tile([1, 64], f32r)

            nc.gpsimd.dma_start(out=bo_sb,
                                in_=bo_d.unsqueeze(0).to_broadcast([128, DM]))
            nc.vector.memset(onef, 1.0)
            nc.vector.tensor_copy(out=ones_sb,
                                  in_=onef[0:1, 0:1].to_broadcast([1, 64]))

            dram = ctx.enter_context(
                tc.tile_pool(name="dram", bufs=1, space="DRAM"))
            KT_dram = dram.tile([INNER, N], f32r)   # K^T [inner, keys]
            V_dram = dram.tile([N, INNER], f32r)    # V   [keys, inner]

            xT_r = xT_d.rearrange("(a p) n -> a p n", p=128)

            # ---------------- Phase A: projections ----------------
            with tc.tile_pool(name="pa_x", bufs=1) as pa_x:
                xT_sb = pa_x.tile([128, A, N], f32r)
                for a in range(A):
                    nc.sync.dma_start(out=xT_sb[:, a, :], in_=xT_r[a])

                # --- Q^T (query cols = 0:512 of rotated x^T) ---
                with tc.tile_pool(name="pa_wq", bufs=1) as pa_wq, \
                     tc.tile_pool(name="pa_ps1", bufs=4, space="PSUM") as ps1:
                    Wq_sb = pa_wq.tile([128, A, INNER], f32r)
                    Wq_r = Wq_d.rearrange("(a p) i -> a p i", p=128)
                    for a in range(A):
                        nc.sync.dma_start(out=Wq_sb[:, a, :], in_=Wq_r[a])
                    for ib in range(IB):
                        qp = ps1.tile([128, QR], f32, tag="proj")
                        for a in range(A):
                            nc.tensor.matmul(
                                out=qp,
                                lhsT=Wq_sb[:, a, ib * 128:(ib + 1) * 128],
                                rhs=xT_sb[:, a, 0:QR],
                                start=(a == 0), stop=(a == A - 1))
                        nc.vector.tensor_copy(out=QT_sb[:, ib, :], in_=qp)

                # --- K^T and V, staged to DRAM ---
                with tc.tile_pool(name="pa_wkv", bufs=1) as pa_wkv, \
                     tc.tile_pool(name="pa_stage", bufs=4) as pstg, \
                     tc.tile_pool(name="pa_ps2", bufs=4, space="PSUM") as ps2:
                    Wkv_sb = pa_wkv.tile([128, A, 2 * INNER], f32r)
                    Wkv_r = Wkv_d.rearrange("(a p) i -> a p i", p=128)
                    for a in range(A):
                        nc.sync.dma_start(out=Wkv_sb[:, a, :], in_=Wkv_r[a])

                    for ib in range(IB):
                        for kc in range(KC):
                            kp = ps2.tile([128, 512], f32, tag="proj")
                            for a in range(A):
                                nc.tensor.matmul(
                                    out=kp,
                                    lhsT=Wkv_sb[:, a, ib * 128:(ib + 1) * 128],
                                    rhs=xT_sb[:, a, kc * 512:(kc + 1) * 512],
                                    start=(a == 0), stop=(a == A - 1))
                            kstg = pstg.tile([128, 512], f32r, tag="stage")
                            nc.vector.tensor_copy(out=kstg, in_=kp)
                            nc.sync.dma_start(
                                out=KT_dram[ib * 128:(ib + 1) * 128,
                                            kc * 512:(kc + 1) * 512],
                                in_=kstg)

                    for kb in range(KB):
                        for ic in range(2):
                            vp = ps2.tile([128, 512], f32, tag="proj")
                            for a in range(A):
                                nc.tensor.matmul(
                                    out=vp,
                                    lhsT=xT_sb[:, a, kb * 128:(kb + 1) * 128],
                                    rhs=Wkv_sb[:, a, INNER + ic * 512:
                                               INNER + (ic + 1) * 512],
                                    start=(a == 0), stop=(a == A - 1))
                            vstg = pstg.tile([128, 512], f32r, tag="stage")
                            nc.vector.tensor_copy(out=vstg, in_=vp)
                            nc.sync.dma_start(
                                out=V_dram[kb * 128:(kb + 1) * 128,
                                           ic * 512:(ic + 1) * 512],
                                in_=vstg)

            # ---------------- Phases B + C ----------------
            with tc.tile_pool(name="pb_wo", bufs=1) as pwo:
                Wo_sb = pwo.tile([128, IB, DM], f32r)
                Wo_r = Wo_d.rearrange("(ib p) d -> ib p d", p=128)
                for ib in range(IB):
                    nc.sync.dma_start(out=Wo_sb[:, ib, :], in_=Wo_r[ib])

                V_r = V_dram.rearrange("(kb p) i -> p kb i", p=128)

                # --- Phase B: per-head attention ---
                with tc.tile_pool(name="pb_kt", bufs=2) as pkt, \
                     tc.tile_pool(name="pb_v", bufs=2) as pv, \
                     tc.tile_pool(name="pb_es", bufs=2) as pes, \
                     tc.tile_pool(name="pb_ps", bufs=1, space="PSUM") as psb:
                    for hp in range(H // 2):
                        KT_pair = pkt.tile([128, N], f32r, tag="kt")
                        nc.sync.dma_start(
                            out=KT_pair,
                            in_=KT_dram[hp * 128:(hp + 1) * 128, :])
                        for hh in range(2):
                            h = hp * 2 + hh
                            V_aug = pv.tile([128, KB, 65], f32r, tag="vaug")
                            nc.sync.dma_start(
                                out=V_aug[:, :, 0:64],
                                in_=V_r[:, :, h * 64:(h + 1) * 64])
                            nc.vector.tensor_copy(
                                out=V_aug[:, :, 64:65],
                                in_=onef.unsqueeze(1).to_broadcast([128, KB, 1]))

                            expS = pes.tile([128, KB, QR], f32r, tag="es")
                            op = psb.tile([65, QR], f32, tag="o", bufs=2)
                            for kb in range(KB):
                                sp = psb.tile([128, QR], f32, tag="s", bufs=3)
                                nc.tensor.matmul(
                                    out=sp,
                                    lhsT=KT_pair[hh * 64:(hh + 1) * 64,
                                                 kb * 128:(kb + 1) * 128],
                                    rhs=QT_sb[hh * 64:(hh + 1) * 64, hp, :],
                                    start=True, stop=True)
                                nc.scalar.activation(
                                    out=expS[:, kb, :], in_=sp, func=Exp,
                                    scale=SCALE)
                                nc.tensor.matmul(
                                    out=op,
                                    lhsT=V_aug[:, kb, :],
                                    rhs=expS[:, kb, :],
                                    start=(kb == 0), stop=(kb == KB - 1))
                            recip = pv.tile([1, QR], f32r, tag="recip")
                            nc.vector.reciprocal(out=recip, in_=op[64:65, :])
                            rb = psb.tile([64, QR], f32, tag="rb", bufs=2)
                            nc.tensor.matmul(
                                out=rb, lhsT=ones_sb, rhs=recip,
                                start=True, stop=True)
                            rbs = pv.tile([64, QR], f32, tag="rbs")
                            nc.vector.tensor_copy(out=rbs, in_=rb)
                            nc.vector.tensor_mul(
                                OT_sb[hh * 64:(hh + 1) * 64, hp, :],
                                op[0:64, :], rbs)

                # --- Phase C: output projection ---
                with tc.tile_pool(name="pc_out", bufs=4) as pout, \
                     tc.tile_pool(name="pc_ps", bufs=4, space="PSUM") as psc:
                    for qb in range(QB):
                        for dc in range(2):
                            outp = psc.tile([128, 512], f32, tag="out")
                            for ib in range(IB):
                                nc.tensor.matmul(
                                    out=outp,
                                    lhsT=OT_sb[:, ib, qb * 128:(qb + 1) * 128],
                                    rhs=Wo_sb[:, ib, dc * 512:(dc + 1) * 512],
                                    start=(ib == 0), stop=(ib == IB - 1))
                            ob = pout.tile([128, 512], f32, tag="ob")
                            nc.vector.tensor_add(
                                ob, outp, bo_sb[:, dc * 512:(dc + 1) * 512])
                            nc.sync.dma_start(
                                out=out_d[qb * 128:(qb + 1) * 128,
                                          dc * 512:(dc + 1) * 512],
                                in_=ob)

    nc.compile()
    return nc


def _get_nc():
    if "nc" not in _cached:
        _cached["nc"] = _build()
    return _cached["nc"]


def kernel(queries, Wq, Wkv, Wo, bo, _trace=False):
    from concourse.bass_utils import run_bass_kernel_spmd

    queries = np.asarray(queries, dtype=np.float32)
    Wq = np.asarray(Wq, dtype=np.float32)
    Wkv = np.asarray(Wkv, dtype=np.float32)
    Wo = np.asarray(Wo, dtype=np.float32)
    bo = np.asarray(bo, dtype=np.float32)

    nc = _get_nc()

    in_maps = []
    for c in range(NCORES):
        g, r = c // 4, c % 4
        xT = np.ascontiguousarray(queries[g].T)          # [DM, N]
        xT = np.ascontiguousarray(np.roll(xT, -r * QR, axis=1))
        in_maps.append({"xT": xT, "Wq": Wq, "Wkv": Wkv, "Wo": Wo, "bo": bo})

    res = run_bass_kernel_spmd(nc, in_maps, list(range(NCORES)),
                               trace=_trace)
    out = np.empty((B, N, DM), dtype=np.float32)
    for c in range(NCORES):
        g, r = c // 4, c % 4
        out[g, r * QR:(r + 1) * QR, :] = res.results[c]["out"]
    if _trace:
        return out, res
    return out


if __name__ == "__main__":
    rng = np.random.default_rng(0)
    q = rng.standard_normal((B, N, DM), dtype=np.float32)
    s = 0.02
    inputs = dict(
        queries=q,
        Wq=(rng.standard_normal((DM, INNER), dtype=np.float32) * s),
        Wkv=(rng.standard_normal((DM, 2 * INNER), dtype=np.float32) * s),
        Wo=(rng.standard_normal((INNER, DM), dtype=np.float32) * s),
        bo=(rng.standard_normal((DM,), dtype=np.float32) * s),
    )
    out = kernel(**inputs)
    print("kernel ran, out shape", out.shape)


# revision 11
# speedup vs baseline: 1.3598x; 1.3598x over previous
"""Trainium2 Bass kernel for 16-head self-attention (b=2, n=2048, dm=1024, dh=64).

Sharding: each of 8 cores owns (batch g = c//4, sequence block r = c%4).
A core computes K,V for its batch's FULL sequence (replicated across the 4
cores of that batch -- avoids cross-core collectives entirely), attention for
all 16 heads restricted to its 512 query rows, and the output projection for
those rows.  Per-core outputs are disjoint [512, 1024] slices of the final
[2, 2048, 1024]; the host concatenates.

Key layout trick: the host passes x^T (dm-major) ROTATED by the core's row
offset, so every core's query slice is columns 0:512 of its own x^T -- the
SPMD program is identical across cores.  Attention is permutation-invariant
over keys, so the rotation does not change the result.

All matmuls use float32r (4-xbus fp32 streaming: 1 cycle/row when the moving
free dim is >= 256, vs 4 for plain fp32; ~tf32 effective precision, measured
~2e-4 rel err end-to-end).  fp32r only hits full rate on the FULL 128x128
array (measured: K=64 shapes 3.4x slower, M=65 shapes 2.9x slower), so both
attention matmuls are padded to 128x128:
  S^T = (full K^T head-pair as lhsT) @ (zero-padded Q^T as rhs) -- the other
        head's 64 contraction lanes multiply zeros;
  O'' = ([V_h | 1 | 0pad] as lhsT, 128 cols) @ (exp(S^T) as rhs) -- PSUM rows
        65..127 compute zeros and are never read; row 64 = softmax denom.
No on-chip transposes are needed anywhere:
  Q^T[i,q]  = (Wq  as lhsT)  @ (x^T as rhs)
  K^T[i,k]  = (Wk  as lhsT)  @ (x^T as rhs)
  V [k,i]   = (x^T as lhsT)  @ (Wv  as rhs)
  S^T[k,q]  = (K^T as lhsT)  @ (Q^T as rhs)          (per head, dh=64)
  O'' [d,q] = ([V|1|0] as lhsT) @ (exp(S^T) as rhs)  (row 64 = softmax denom)
  out[q,d]  = (O^T as lhsT)  @ (Wo  as rhs) + bo
The 1/denom broadcast across partitions is a K=1 outer-product matmul with a
ones vector (bounced through SBUF: DVE may read only one PSUM operand per
instruction).
"""

import sys

for _p in ("/opt/trn_rl_repo", "/root/.axon_site/_ro/trn_rl_repo"):
    if _p not in sys.path:
        sys.path.append(_p)

import numpy as np

B = 2
N = 2048
DM = 1024
H = 16
DH = 64
INNER = H * DH  # 1024
NCORES = 8
QR = 512  # query rows per core
SCALE = DH ** -0.5

_cached = {}


def _build():
    import contextlib
    import concourse.bacc as bacc
    import concourse.tile as tile
    import concourse.mybir as mybir

    f32 = mybir.dt.float32
    f32r = mybir.dt.float32r
    Exp = mybir.ActivationFunctionType.Exp

    nc = bacc.Bacc("TRN2", target_bir_lowering=False, debug=False,
                   enable_asserts=False)

    xT_d = nc.dram_tensor("xT", [DM, N], f32r, kind="ExternalInput").ap()
    Wq_d = nc.dram_tensor("Wq", [DM, INNER], f32r, kind="ExternalInput").ap()
    Wkv_d = nc.dram_tensor("Wkv", [DM, 2 * INNER], f32r, kind="ExternalInput").ap()
    Wo_d = nc.dram_tensor("Wo", [INNER, DM], f32r, kind="ExternalInput").ap()
    bo_d = nc.dram_tensor("bo", [DM], f32, kind="ExternalInput").ap()
    out_d = nc.dram_tensor("out", [QR, DM], f32, kind="ExternalOutput").ap()

    A = DM // 128      # 8 dm blocks
    IB = INNER // 128  # 8 inner blocks
    KB = N // 128      # 16 key blocks
    KC = N // 512      # 4 key chunks
    QB = QR // 128     # 4 query blocks

    with tile.TileContext(nc) as tc, \
         nc.allow_low_precision(reason="fp32r matmul pipeline, validated e2e"), \
         contextlib.ExitStack() as ctx:
            persist = ctx.enter_context(tc.tile_pool(name="persist", bufs=1))
            # Q^T zero-padded per (head-pair, parity): slot hh holds the
            # head's 64 rows, the other 64 rows stay zero so the S^T matmul
            # can contract over the full 128 partitions at fp32r full rate.
            QT_z = persist.tile([128, IB, 2, QR], f32r)
            OT_sb = persist.tile([128, IB, QR], f32r)   # O^T  [inner, q]
            bo_sb = persist.tile([128, DM], f32)
            onef = persist.tile([128, 1], f32)
            zerof = persist.tile([128, 1], f32)
            ones_sb = persist.tile([1, 64], f32r)

            nc.gpsimd.dma_start(out=bo_sb,
                                in_=bo_d.unsqueeze(0).to_broadcast([128, DM]))
            nc.vector.memset(onef, 1.0)
            nc.vector.memset(zerof, 0.0)
            nc.vector.tensor_copy(out=ones_sb,
                                  in_=onef[0:1, 0:1].to_broadcast([1, 64]))
            # zero the padding halves of QT_z (memset can't write f32r)
            nc.vector.tensor_copy(
                out=QT_z[:, :, :, :],
                in_=zerof.unsqueeze(1).unsqueeze(1).to_broadcast(
                    [128, IB, 2, QR]))

            dram = ctx.enter_context(
                tc.tile_pool(name="dram", bufs=1, space="DRAM"))
            KT_dram = dram.tile([INNER, N], f32r)   # K^T [inner, keys]
            V_dram = dram.tile([N, INNER], f32r)    # V   [keys, inner]

            xT_r = xT_d.rearrange("(a p) n -> a p n", p=128)

            # ---------------- Phase A: projections ----------------
            with tc.tile_pool(name="pa_x", bufs=1) as pa_x:
                xT_sb = pa_x.tile([128, A, N], f32r)
                for a in range(A):
                    nc.sync.dma_start(out=xT_sb[:, a, :], in_=xT_r[a])

                # --- Q^T (query cols = 0:512 of rotated x^T) ---
                with tc.tile_pool(name="pa_wq", bufs=1) as pa_wq, \
                     tc.tile_pool(name="pa_ps1", bufs=4, space="PSUM") as ps1:
                    Wq_sb = pa_wq.tile([128, A, INNER], f32r)
                    Wq_r = Wq_d.rearrange("(a p) i -> a p i", p=128)
                    for a in range(A):
                        nc.sync.dma_start(out=Wq_sb[:, a, :], in_=Wq_r[a])
                    for ib in range(IB):
                        qp = ps1.tile([128, QR], f32, tag="proj")
                        for a in range(A):
                            nc.tensor.matmul(
                                out=qp,
                                lhsT=Wq_sb[:, a, ib * 128:(ib + 1) * 128],
                                rhs=xT_sb[:, a, 0:QR],
                                start=(a == 0), stop=(a == A - 1))
                        nc.vector.tensor_copy(out=QT_z[0:64, ib, 0, :],
                                              in_=qp[0:64, :])
                        nc.vector.tensor_copy(out=QT_z[64:128, ib, 1, :],
                                              in_=qp[64:128, :])

                # --- K^T, staged to DRAM ---
                Wkv_r = Wkv_d.rearrange("(a p) i -> a p i", p=128)
                with tc.tile_pool(name="pa_wk", bufs=1) as pa_wk, \
                     tc.tile_pool(name="pa_stage", bufs=4) as pstg, \
                     tc.tile_pool(name="pa_ps2", bufs=4, space="PSUM") as ps2:
                    Wk_sb = pa_wk.tile([128, A, INNER], f32r)
                    for a in range(A):
                        nc.sync.dma_start(out=Wk_sb[:, a, :],
                                          in_=Wkv_r[a, :, 0:INNER])

                    for ib in range(IB):
                        for kc in range(KC):
                            kp = ps2.tile([128, 512], f32, tag="proj")
                            for a in range(A):
                                nc.tensor.matmul(
                                    out=kp,
                                    lhsT=Wk_sb[:, a, ib * 128:(ib + 1) * 128],
                                    rhs=xT_sb[:, a, kc * 512:(kc + 1) * 512],
                                    start=(a == 0), stop=(a == A - 1))
                            kstg = pstg.tile([128, 512], f32r, tag="stage")
                            nc.vector.tensor_copy(out=kstg, in_=kp)
                            nc.sync.dma_start(
                                out=KT_dram[ib * 128:(ib + 1) * 128,
                                            kc * 512:(kc + 1) * 512],
                                in_=kstg)

                # --- V, staged to DRAM ---
                with tc.tile_pool(name="pa_wv", bufs=1) as pa_wv, \
                     tc.tile_pool(name="pa_stage2", bufs=4) as pstg2, \
                     tc.tile_pool(name="pa_ps3", bufs=4, space="PSUM") as ps3:
                    Wv_sb = pa_wv.tile([128, A, INNER], f32r)
                    for a in range(A):
                        nc.sync.dma_start(out=Wv_sb[:, a, :],
                                          in_=Wkv_r[a, :, INNER:2 * INNER])

                    for kb in range(KB):
                        for ic in range(2):
                            vp = ps3.tile([128, 512], f32, tag="proj")
                            for a in range(A):
                                nc.tensor.matmul(
                                    out=vp,
                                    lhsT=xT_sb[:, a, kb * 128:(kb + 1) * 128],
                                    rhs=Wv_sb[:, a, ic * 512:(ic + 1) * 512],
                                    start=(a == 0), stop=(a == A - 1))
                            vstg = pstg2.tile([128, 512], f32r, tag="stage")
                            nc.vector.tensor_copy(out=vstg, in_=vp)
                            nc.sync.dma_start(
                                out=V_dram[kb * 128:(kb + 1) * 128,
                                           ic * 512:(ic + 1) * 512],
                                in_=vstg)

            # ---------------- Phases B + C ----------------
            with tc.tile_pool(name="pb_wo", bufs=1) as pwo:
                Wo_sb = pwo.tile([128, IB, DM], f32r)
                Wo_r = Wo_d.rearrange("(ib p) d -> ib p d", p=128)
                for ib in range(IB):
                    nc.sync.dma_start(out=Wo_sb[:, ib, :], in_=Wo_r[ib])

                V_r = V_dram.rearrange("(kb p) i -> p kb i", p=128)

                # --- Phase B: per-head attention ---
                with tc.tile_pool(name="pb_kt", bufs=2) as pkt, \
                     tc.tile_pool(name="pb_v", bufs=2) as pv, \
                     tc.tile_pool(name="pb_es", bufs=2) as pes, \
                     tc.tile_pool(name="pb_ps", bufs=1, space="PSUM") as psb:
                    for hp in range(H // 2):
                        KT_pair = pkt.tile([128, N], f32r, tag="kt")
                        nc.sync.dma_start(
                            out=KT_pair,
                            in_=KT_dram[hp * 128:(hp + 1) * 128, :])
                        for hh in range(2):
                            h = hp * 2 + hh
                            # [V_h | 1 | 0pad] -> full-width (M=128) lhsT
                            V_aug = pv.tile([128, KB, 128], f32r, tag="vaug")
                            nc.sync.dma_start(
                                out=V_aug[:, :, 0:64],
                                in_=V_r[:, :, h * 64:(h + 1) * 64])
                            nc.vector.tensor_copy(
                                out=V_aug[:, :, 64:65],
                                in_=onef.unsqueeze(1).to_broadcast([128, KB, 1]))
                            nc.vector.tensor_copy(
                                out=V_aug[:, :, 65:128],
                                in_=zerof.unsqueeze(1).to_broadcast(
                                    [128, KB, 63]))

                            expS = pes.tile([128, KB, QR], f32r, tag="es")
                            op = psb.tile([128, QR], f32, tag="o", bufs=2)
                            for kb in range(KB):
                                sp = psb.tile([128, QR], f32, tag="s", bufs=3)
                                nc.tensor.matmul(
                                    out=sp,
                                    lhsT=KT_pair[:, kb * 128:(kb + 1) * 128],
                                    rhs=QT_z[:, hp, hh, :],
                                    start=True, stop=True)
                                nc.scalar.activation(
                                    out=expS[:, kb, :], in_=sp, func=Exp,
                                    scale=SCALE)
                                nc.tensor.matmul(
                                    out=op,
                                    lhsT=V_aug[:, kb, :],
                                    rhs=expS[:, kb, :],
                                    start=(kb == 0), stop=(kb == KB - 1))
                            recip = pv.tile([1, QR], f32r, tag="recip")
                            nc.vector.reciprocal(out=recip, in_=op[64:65, :])
                            rb = psb.tile([64, QR], f32, tag="rb", bufs=2)
                            nc.tensor.matmul(
                                out=rb, lhsT=ones_sb, rhs=recip,
                                start=True, stop=True)
                            rbs = pv.tile([64, QR], f32, tag="rbs")
                            nc.vector.tensor_copy(out=rbs, in_=rb)
                            nc.vector.tensor_mul(
                                OT_sb[hh * 64:(hh + 1) * 64, hp, :],
                                op[0:64, :], rbs)

                # --- Phase C: output projection ---
                with tc.tile_pool(name="pc_out", bufs=4) as pout, \
                     tc.tile_pool(name="pc_ps", bufs=4, space="PSUM") as psc:
                    for qb in range(QB):
                        for dc in range(2):
                            outp = psc.tile([128, 512], f32, tag="out")
                            for ib in range(IB):
                                nc.tensor.matmul(
                                    out=outp,
                                    lhsT=OT_sb[:, ib, qb * 128:(qb + 1) * 128],
                                    rhs=Wo_sb[:, ib, dc * 512:(dc + 1) * 512],
                                    start=(ib == 0), stop=(ib == IB - 1))
                            ob = pout.tile([128, 512], f32, tag="ob")
                            nc.vector.tensor_add(
                                ob, outp, bo_sb[:, dc * 512:(dc + 1) * 512])
                            nc.sync.dma_start(
                                out=out_d[qb * 128:(qb + 1) * 128,
                                          dc * 512:(dc + 1) * 512],
                                in_=ob)

    nc.compile()
    return nc


def _get_nc():
    if "nc" not in _cached:
        _cached["nc"] = _build()
    return _cached["nc"]


def kernel(queries, Wq, Wkv, Wo, bo, _trace=False):
    from concourse.bass_utils import run_bass_kernel_spmd

    queries = np.asarray(queries, dtype=np.float32)
    Wq = np.asarray(Wq, dtype=np.float32)
    Wkv = np.asarray(Wkv, dtype=np.float32)
    Wo = np.asarray(Wo, dtype=np.float32)
    bo = np.asarray(bo, dtype=np.float32)

    nc = _get_nc()

    in_maps = []
    for c in range(NCORES):
        g, r = c // 4, c % 4
        xT = np.ascontiguousarray(queries[g].T)          # [DM, N]
        xT = np.ascontiguousarray(np.roll(xT, -r * QR, axis=1))
        in_maps.append({"xT": xT, "Wq": Wq, "Wkv": Wkv, "Wo": Wo, "bo": bo})

    res = run_bass_kernel_spmd(nc, in_maps, list(range(NCORES)),
                               trace=_trace)
    out = np.empty((B, N, DM), dtype=np.float32)
    for c in range(NCORES):
        g, r = c // 4, c % 4
        out[g, r * QR:(r + 1) * QR, :] = res.results[c]["out"]
    if _trace:
        return out, res
    return out


if __name__ == "__main__":
    rng = np.random.default_rng(0)
    q = rng.standard_normal((B, N, DM), dtype=np.float32)
    s = 0.02
    inputs = dict(
        queries=q,
        Wq=(rng.standard_normal((DM, INNER), dtype=np.float32) * s),
        Wkv=(rng.standard_normal((DM, 2 * INNER), dtype=np.float32) * s),
        Wo=(rng.standard_normal((INNER, DM), dtype=np.float32) * s),
        bo=(rng.standard_normal((DM,), dtype=np.float32) * s),
    )
    out = kernel(**inputs)
    print("kernel ran, out shape", out.shape)


# revision 16
# speedup vs baseline: 1.5411x; 1.1333x over previous
"""Trainium2 Bass kernel for 16-head self-attention (b=2, n=2048, dm=1024, dh=64).

Sharding: each of 8 cores owns (batch g = c//4, sequence block r = c%4).
A core computes K,V for its batch's FULL sequence (replicated across the 4
cores of that batch -- avoids cross-core collectives entirely), attention for
all 16 heads restricted to its 512 query rows, and the output projection for
those rows.  Per-core outputs are disjoint [512, 1024] slices of the final
[2, 2048, 1024]; the host concatenates.

Key layout trick: the host passes x^T (dm-major) ROTATED by the core's row
offset, so every core's query slice is columns 0:512 of its own x^T -- the
SPMD program is identical across cores.  Attention is permutation-invariant
over keys, so the rotation does not change the result.

All matmuls use float32r (4-xbus fp32 streaming: 1 cycle/row when the moving
free dim is >= 256, vs 4 for plain fp32; ~tf32 effective precision, measured
~2e-4 rel err end-to-end).  fp32r only hits full rate on the FULL 128x128
array (measured: K=64 shapes 3.4x slower, M=65 shapes 2.9x slower), so both
attention matmuls are padded to 128x128:
  S^T = (full K^T head-pair as lhsT) @ (zero-padded Q^T as rhs) -- the other
        head's 64 contraction lanes multiply zeros;
  O'' = ([V_h | 1 | 0pad] as lhsT, 128 cols) @ (exp(S^T) as rhs) -- PSUM rows
        65..127 compute zeros and are never read; row 64 = softmax denom.
No on-chip transposes are needed anywhere:
  Q^T[i,q]  = (Wq  as lhsT)  @ (x^T as rhs)
  K^T[i,k]  = (Wk  as lhsT)  @ (x^T as rhs)
  V [k,i]   = (x^T as lhsT)  @ (Wv  as rhs)
  S^T[k,q]  = (K^T as lhsT)  @ (Q^T as rhs)          (per head, dh=64)
  O'' [d,q] = ([V|1|0] as lhsT) @ (exp(S^T) as rhs)  (row 64 = softmax denom)
  out[q,d]  = (O^T as lhsT)  @ (Wo  as rhs) + bo
The 1/denom broadcast across partitions is an SBUF->SBUF DMA with a
partition-stride-0 read (engines cannot broadcast across partitions, and a
PE outer-product broadcast would stall the in-order PE stream on the DVE
reciprocal).
"""

import sys

for _p in ("/opt/trn_rl_repo", "/root/.axon_site/_ro/trn_rl_repo"):
    if _p not in sys.path:
        sys.path.append(_p)

import numpy as np

B = 2
N = 2048
DM = 1024
H = 16
DH = 64
INNER = H * DH  # 1024
NCORES = 8
QR = 512  # query rows per core
SCALE = DH ** -0.5

_cached = {}


def _build():
    import contextlib
    import concourse.bacc as bacc
    import concourse.tile as tile
    import concourse.mybir as mybir

    f32 = mybir.dt.float32
    f32r = mybir.dt.float32r
    Exp = mybir.ActivationFunctionType.Exp

    nc = bacc.Bacc("TRN2", target_bir_lowering=False, debug=False,
                   enable_asserts=False)

    xT_d = nc.dram_tensor("xT", [DM, N], f32r, kind="ExternalInput").ap()
    Wq_d = nc.dram_tensor("Wq", [DM, INNER], f32r, kind="ExternalInput").ap()
    Wkv_d = nc.dram_tensor("Wkv", [DM, 2 * INNER], f32r, kind="ExternalInput").ap()
    Wo_d = nc.dram_tensor("Wo", [INNER, DM], f32r, kind="ExternalInput").ap()
    bo_d = nc.dram_tensor("bo", [DM], f32, kind="ExternalInput").ap()
    out_d = nc.dram_tensor("out", [QR, DM], f32, kind="ExternalOutput").ap()

    A = DM // 128      # 8 dm blocks
    IB = INNER // 128  # 8 inner blocks
    KB = N // 128      # 16 key blocks
    KC = N // 512      # 4 key chunks
    QB = QR // 128     # 4 query blocks

    with tile.TileContext(nc) as tc, \
         nc.allow_low_precision(reason="fp32r matmul pipeline, validated e2e"), \
         contextlib.ExitStack() as ctx:
            persist = ctx.enter_context(tc.tile_pool(name="persist", bufs=1))
            # Q^T zero-padded per (head-pair, parity): slot hh holds the
            # head's 64 rows, the other 64 rows stay zero so the S^T matmul
            # can contract over the full 128 partitions at fp32r full rate.
            QT_z = persist.tile([128, IB, 2, QR], f32r)
            OT_sb = persist.tile([128, IB, QR], f32r)   # O^T  [inner, q]
            bo_sb = persist.tile([128, DM], f32)
            onef = persist.tile([128, 1], f32)
            zerof = persist.tile([128, 1], f32)

            nc.gpsimd.dma_start(out=bo_sb,
                                in_=bo_d.unsqueeze(0).to_broadcast([128, DM]))
            nc.vector.memset(onef, 1.0)
            nc.vector.memset(zerof, 0.0)
            # zero the padding halves of QT_z (memset can't write f32r)
            nc.vector.tensor_copy(
                out=QT_z[:, :, :, :],
                in_=zerof.unsqueeze(1).unsqueeze(1).to_broadcast(
                    [128, IB, 2, QR]))

            dram = ctx.enter_context(
                tc.tile_pool(name="dram", bufs=1, space="DRAM"))
            KT_dram = dram.tile([INNER, N], f32r)   # K^T [inner, keys]
            V_dram = dram.tile([N, INNER], f32r)    # V   [keys, inner]
            dram2 = ctx.enter_context(
                tc.tile_pool(name="dram2", bufs=4, space="DRAM"))

            xT_r = xT_d.rearrange("(a p) n -> a p n", p=128)

            # ---------------- Phase A: projections ----------------
            with tc.tile_pool(name="pa_x", bufs=1) as pa_x:
                xT_sb = pa_x.tile([128, A, N], f32r)
                for a in range(A):
                    nc.sync.dma_start(out=xT_sb[:, a, :], in_=xT_r[a])

                # --- Q^T (query cols = 0:512 of rotated x^T) ---
                with tc.tile_pool(name="pa_wq", bufs=1) as pa_wq, \
                     tc.tile_pool(name="pa_ps1", bufs=4, space="PSUM") as ps1:
                    Wq_sb = pa_wq.tile([128, A, INNER], f32r)
                    Wq_r = Wq_d.rearrange("(a p) i -> a p i", p=128)
                    for a in range(A):
                        nc.sync.dma_start(out=Wq_sb[:, a, :], in_=Wq_r[a])
                    for ib in range(IB):
                        qp = ps1.tile([128, QR], f32, tag="proj")
                        for a in range(A):
                            nc.tensor.matmul(
                                out=qp,
                                lhsT=Wq_sb[:, a, ib * 128:(ib + 1) * 128],
                                rhs=xT_sb[:, a, 0:QR],
                                start=(a == 0), stop=(a == A - 1))
                        nc.vector.tensor_copy(out=QT_z[0:64, ib, 0, :],
                                              in_=qp[0:64, :])
                        nc.vector.tensor_copy(out=QT_z[64:128, ib, 1, :],
                                              in_=qp[64:128, :])

                # --- K^T, staged to DRAM ---
                Wkv_r = Wkv_d.rearrange("(a p) i -> a p i", p=128)
                with tc.tile_pool(name="pa_wk", bufs=1) as pa_wk, \
                     tc.tile_pool(name="pa_stage", bufs=4) as pstg, \
                     tc.tile_pool(name="pa_ps2", bufs=4, space="PSUM") as ps2:
                    Wk_sb = pa_wk.tile([128, A, INNER], f32r)
                    for a in range(A):
                        nc.sync.dma_start(out=Wk_sb[:, a, :],
                                          in_=Wkv_r[a, :, 0:INNER])

                    for ib in range(IB):
                        for kc in range(KC):
                            kp = ps2.tile([128, 512], f32, tag="proj")
                            for a in range(A):
                                nc.tensor.matmul(
                                    out=kp,
                                    lhsT=Wk_sb[:, a, ib * 128:(ib + 1) * 128],
                                    rhs=xT_sb[:, a, kc * 512:(kc + 1) * 512],
                                    start=(a == 0), stop=(a == A - 1))
                            kstg = pstg.tile([128, 512], f32r, tag="stage")
                            nc.vector.tensor_copy(out=kstg, in_=kp)
                            nc.sync.dma_start(
                                out=KT_dram[ib * 128:(ib + 1) * 128,
                                            kc * 512:(kc + 1) * 512],
                                in_=kstg)

                # --- V, staged to DRAM ---
                with tc.tile_pool(name="pa_wv", bufs=1) as pa_wv, \
                     tc.tile_pool(name="pa_stage2", bufs=4) as pstg2, \
                     tc.tile_pool(name="pa_ps3", bufs=4, space="PSUM") as ps3:
                    Wv_sb = pa_wv.tile([128, A, INNER], f32r)
                    for a in range(A):
                        nc.sync.dma_start(out=Wv_sb[:, a, :],
                                          in_=Wkv_r[a, :, INNER:2 * INNER])

                    # ic-major so V columns for the first head pairs land in
                    # DRAM as early as possible (phase B consumes per-head
                    # column slices across all key blocks).
                    for ic in range(2):
                        for kb in range(KB):
                            vp = ps3.tile([128, 512], f32, tag="proj")
                            for a in range(A):
                                nc.tensor.matmul(
                                    out=vp,
                                    lhsT=xT_sb[:, a, kb * 128:(kb + 1) * 128],
                                    rhs=Wv_sb[:, a, ic * 512:(ic + 1) * 512],
                                    start=(a == 0), stop=(a == A - 1))
                            vstg = pstg2.tile([128, 512], f32r, tag="stage")
                            nc.vector.tensor_copy(out=vstg, in_=vp)
                            nc.sync.dma_start(
                                out=V_dram[kb * 128:(kb + 1) * 128,
                                           ic * 512:(ic + 1) * 512],
                                in_=vstg)

            # ---------------- Phases B + C ----------------
            with tc.tile_pool(name="pb_wo", bufs=1) as pwo:
                Wo_sb = pwo.tile([128, IB, DM], f32r)
                Wo_r = Wo_d.rearrange("(ib p) d -> ib p d", p=128)
                for ib in range(IB):
                    nc.sync.dma_start(out=Wo_sb[:, ib, :], in_=Wo_r[ib])

                V_r = V_dram.rearrange("(kb p) i -> p kb i", p=128)

                # --- Phase B: per-head attention ---
                with tc.tile_pool(name="pb_kt", bufs=2) as pkt, \
                     tc.tile_pool(name="pb_v", bufs=2) as pv, \
                     tc.tile_pool(name="pb_es", bufs=2) as pes, \
                     tc.tile_pool(name="pb_ps", bufs=1, space="PSUM") as psb:
                    for hp in range(H // 2):
                        KT_pair = pkt.tile([128, N], f32r, tag="kt")
                        nc.sync.dma_start(
                            out=KT_pair,
                            in_=KT_dram[hp * 128:(hp + 1) * 128, :])
                        for hh in range(2):
                            h = hp * 2 + hh
                            # [V_h | 1 | 0pad] -> full-width (M=128) lhsT
                            V_aug = pv.tile([128, KB, 128], f32r, tag="vaug")
                            nc.sync.dma_start(
                                out=V_aug[:, :, 0:64],
                                in_=V_r[:, :, h * 64:(h + 1) * 64])
                            nc.vector.tensor_copy(
                                out=V_aug[:, :, 64:65],
                                in_=onef.unsqueeze(1).to_broadcast([128, KB, 1]))
                            nc.vector.tensor_copy(
                                out=V_aug[:, :, 65:128],
                                in_=zerof.unsqueeze(1).to_broadcast(
                                    [128, KB, 63]))

                            expS = pes.tile([128, KB, QR], f32r, tag="es")
                            op = psb.tile([128, QR], f32, tag="o", bufs=2)
                            for kb in range(KB):
                                sp = psb.tile([128, QR], f32, tag="s", bufs=3)
                                nc.tensor.matmul(
                                    out=sp,
                                    lhsT=KT_pair[:, kb * 128:(kb + 1) * 128],
                                    rhs=QT_z[:, hp, hh, :],
                                    start=True, stop=True)
                                nc.scalar.activation(
                                    out=expS[:, kb, :], in_=sp, func=Exp,
                                    scale=SCALE)
                                nc.tensor.matmul(
                                    out=op,
                                    lhsT=V_aug[:, kb, :],
                                    rhs=expS[:, kb, :],
                                    start=(kb == 0), stop=(kb == KB - 1))
                            # 1/rowsum, broadcast across 64 partitions via a
                            # DRAM bounce (SBUF DMA reads can't be
                            # partition-stride-0; DRAM reads can).  Keeps the
                            # PE stream free of normalization dependencies.
                            recip = pv.tile([1, QR], f32, tag="recip")
                            nc.vector.reciprocal(out=recip, in_=op[64:65, :])
                            rcd = dram2.tile([1, QR], f32, tag="rcd")
                            nc.sync.dma_start(out=rcd, in_=recip)
                            rbs = pv.tile([64, QR], f32, tag="rbs")
                            nc.sync.dma_start(
                                out=rbs, in_=rcd.to_broadcast([64, QR]))
                            nc.vector.tensor_mul(
                                OT_sb[hh * 64:(hh + 1) * 64, hp, :],
                                op[0:64, :], rbs)

                # --- Phase C: output projection ---
                with tc.tile_pool(name="pc_out", bufs=4) as pout, \
                     tc.tile_pool(name="pc_ps", bufs=4, space="PSUM") as psc:
                    for qb in range(QB):
                        for dc in range(2):
                            outp = psc.tile([128, 512], f32, tag="out")
                            for ib in range(IB):
                                nc.tensor.matmul(
                                    out=outp,
                                    lhsT=OT_sb[:, ib, qb * 128:(qb + 1) * 128],
                                    rhs=Wo_sb[:, ib, dc * 512:(dc + 1) * 512],
                                    start=(ib == 0), stop=(ib == IB - 1))
                            ob = pout.tile([128, 512], f32, tag="ob")
                            nc.vector.tensor_add(
                                ob, outp, bo_sb[:, dc * 512:(dc + 1) * 512])
                            nc.sync.dma_start(
                                out=out_d[qb * 128:(qb + 1) * 128,
                                          dc * 512:(dc + 1) * 512],
                                in_=ob)

    nc.compile()
    return nc


def _get_nc():
    if "nc" not in _cached:
        _cached["nc"] = _build()
    return _cached["nc"]


def kernel(queries, Wq, Wkv, Wo, bo, _trace=False):
    from concourse.bass_utils import run_bass_kernel_spmd

    queries = np.asarray(queries, dtype=np.float32)
    Wq = np.asarray(Wq, dtype=np.float32)
    Wkv = np.asarray(Wkv, dtype=np.float32)
    Wo = np.asarray(Wo, dtype=np.float32)
    bo = np.asarray(bo, dtype=np.float32)

    nc = _get_nc()

    in_maps = []
    for c in range(NCORES):
        g, r = c // 4, c % 4
        xT = np.ascontiguousarray(queries[g].T)          # [DM, N]
        xT = np.ascontiguousarray(np.roll(xT, -r * QR, axis=1))
        in_maps.append({"xT": xT, "Wq": Wq, "Wkv": Wkv, "Wo": Wo, "bo": bo})

    res = run_bass_kernel_spmd(nc, in_maps, list(range(NCORES)),
                               trace=_trace)
    out = np.empty((B, N, DM), dtype=np.float32)
    for c in range(NCORES):
        g, r = c // 4, c % 4
        out[g, r * QR:(r + 1) * QR, :] = res.results[c]["out"]
    if _trace:
        return out, res
    return out


if __name__ == "__main__":
    rng = np.random.default_rng(0)
    q = rng.standard_normal((B, N, DM), dtype=np.float32)
    s = 0.02
    inputs = dict(
        queries=q,
        Wq=(rng.standard_normal((DM, INNER), dtype=np.float32) * s),
        Wkv=(rng.standard_normal((DM, 2 * INNER), dtype=np.float32) * s),
        Wo=(rng.standard_normal((INNER, DM), dtype=np.float32) * s),
        bo=(rng.standard_normal((DM,), dtype=np.float32) * s),
    )
    out = kernel(**inputs)
    print("kernel ran, out shape", out.shape)


# revision 17
# speedup vs baseline: 1.6894x; 1.0962x over previous
"""Trainium2 Bass kernel for 16-head self-attention (b=2, n=2048, dm=1024, dh=64).

Sharding: each of 8 cores owns (batch g = c//4, sequence block r = c%4).
A core computes K,V for its batch's FULL sequence (replicated across the 4
cores of that batch -- avoids cross-core collectives entirely), attention for
all 16 heads restricted to its 512 query rows, and the output projection for
those rows.  Per-core outputs are disjoint [512, 1024] slices of the final
[2, 2048, 1024]; the host concatenates.

Key layout trick: the host passes x^T (dm-major) ROTATED by the core's row
offset, so every core's query slice is columns 0:512 of its own x^T -- the
SPMD program is identical across cores.  Attention is permutation-invariant
over keys, so the rotation does not change the result.

All matmuls use float32r (4-xbus fp32 streaming: 1 cycle/row when the moving
free dim is >= 256, vs 4 for plain fp32; ~tf32 effective precision, measured
~2e-4 rel err end-to-end).  fp32r only hits full rate on the FULL 128x128
array (measured: K=64 shapes 3.4x slower, M=65 shapes 2.9x slower), so both
attention matmuls are padded to 128x128:
  S^T = (full K^T head-pair as lhsT) @ (zero-padded Q^T as rhs) -- the other
        head's 64 contraction lanes multiply zeros;
  O'' = ([V_h | 1 | 0pad] as lhsT, 128 cols) @ (exp(S^T) as rhs) -- PSUM rows
        65..127 compute zeros and are never read; row 64 = softmax denom.
No on-chip transposes are needed anywhere:
  Q^T[i,q]  = (Wq  as lhsT)  @ (x^T as rhs)
  K^T[i,k]  = (Wk  as lhsT)  @ (x^T as rhs)
  V [k,i]   = (x^T as lhsT)  @ (Wv  as rhs)
  S^T[k,q]  = (K^T as lhsT)  @ (Q^T as rhs)          (per head, dh=64)
  O'' [d,q] = ([V|1|0] as lhsT) @ (exp(S^T) as rhs)  (row 64 = softmax denom)
  out[q,d]  = (O^T as lhsT)  @ (Wo  as rhs) + bo
The 1/denom broadcast across partitions is an SBUF->SBUF DMA with a
partition-stride-0 read (engines cannot broadcast across partitions, and a
PE outer-product broadcast would stall the in-order PE stream on the DVE
reciprocal).
"""

import sys

for _p in ("/opt/trn_rl_repo", "/root/.axon_site/_ro/trn_rl_repo"):
    if _p not in sys.path:
        sys.path.append(_p)

import numpy as np

B = 2
N = 2048
DM = 1024
H = 16
DH = 64
INNER = H * DH  # 1024
NCORES = 8
QR = 512  # query rows per core
SCALE = DH ** -0.5

_cached = {}


def _build(mm_dtype="f32r"):
    import contextlib
    import concourse.bacc as bacc
    import concourse.tile as tile
    import concourse.mybir as mybir

    f32 = mybir.dt.float32
    f32r = mybir.dt.bfloat16 if mm_dtype == "bf16" else mybir.dt.float32r
    Exp = mybir.ActivationFunctionType.Exp

    nc = bacc.Bacc("TRN2", target_bir_lowering=False, debug=False,
                   enable_asserts=False)

    xT_d = nc.dram_tensor("xT", [DM, N], f32r, kind="ExternalInput").ap()
    Wq_d = nc.dram_tensor("Wq", [DM, INNER], f32r, kind="ExternalInput").ap()
    Wkv_d = nc.dram_tensor("Wkv", [DM, 2 * INNER], f32r, kind="ExternalInput").ap()
    Wo_d = nc.dram_tensor("Wo", [INNER, DM], f32r, kind="ExternalInput").ap()
    bo_d = nc.dram_tensor("bo", [DM], f32, kind="ExternalInput").ap()
    out_d = nc.dram_tensor("out", [QR, DM], f32, kind="ExternalOutput").ap()

    A = DM // 128      # 8 dm blocks
    IB = INNER // 128  # 8 inner blocks
    KB = N // 128      # 16 key blocks
    KC = N // 512      # 4 key chunks
    QB = QR // 128     # 4 query blocks

    with tile.TileContext(nc) as tc, \
         nc.allow_low_precision(reason="fp32r matmul pipeline, validated e2e"), \
         contextlib.ExitStack() as ctx:
            persist = ctx.enter_context(tc.tile_pool(name="persist", bufs=1))
            # Q^T zero-padded per (head-pair, parity): slot hh holds the
            # head's 64 rows, the other 64 rows stay zero so the S^T matmul
            # can contract over the full 128 partitions at fp32r full rate.
            QT_z = persist.tile([128, IB, 2, QR], f32r)
            OT_sb = persist.tile([128, IB, QR], f32r)   # O^T  [inner, q]
            bo_sb = persist.tile([128, DM], f32)
            onef = persist.tile([128, 1], f32)
            zerof = persist.tile([128, 1], f32)

            nc.gpsimd.dma_start(out=bo_sb,
                                in_=bo_d.unsqueeze(0).to_broadcast([128, DM]))
            nc.vector.memset(onef, 1.0)
            nc.vector.memset(zerof, 0.0)
            # zero the padding halves of QT_z (memset can't write f32r)
            nc.vector.tensor_copy(
                out=QT_z[:, :, :, :],
                in_=zerof.unsqueeze(1).unsqueeze(1).to_broadcast(
                    [128, IB, 2, QR]))

            dram = ctx.enter_context(
                tc.tile_pool(name="dram", bufs=1, space="DRAM"))
            KT_dram = dram.tile([INNER, N], f32r)   # K^T [inner, keys]
            V_dram = dram.tile([N, INNER], f32r)    # V   [keys, inner]
            dram2 = ctx.enter_context(
                tc.tile_pool(name="dram2", bufs=4, space="DRAM"))

            xT_r = xT_d.rearrange("(a p) n -> a p n", p=128)

            # ---------------- Phase A: projections ----------------
            with tc.tile_pool(name="pa_x", bufs=1) as pa_x:
                xT_sb = pa_x.tile([128, A, N], f32r)
                for a in range(A):
                    nc.sync.dma_start(out=xT_sb[:, a, :], in_=xT_r[a])

                # --- Q^T (query cols = 0:512 of rotated x^T) ---
                with tc.tile_pool(name="pa_wq", bufs=1) as pa_wq, \
                     tc.tile_pool(name="pa_ps1", bufs=4, space="PSUM") as ps1:
                    Wq_sb = pa_wq.tile([128, A, INNER], f32r)
                    Wq_r = Wq_d.rearrange("(a p) i -> a p i", p=128)
                    for a in range(A):
                        nc.sync.dma_start(out=Wq_sb[:, a, :], in_=Wq_r[a])
                    for ib in range(IB):
                        qp = ps1.tile([128, QR], f32, tag="proj")
                        for a in range(A):
                            nc.tensor.matmul(
                                out=qp,
                                lhsT=Wq_sb[:, a, ib * 128:(ib + 1) * 128],
                                rhs=xT_sb[:, a, 0:QR],
                                start=(a == 0), stop=(a == A - 1))
                        nc.vector.tensor_copy(out=QT_z[0:64, ib, 0, :],
                                              in_=qp[0:64, :])
                        nc.vector.tensor_copy(out=QT_z[64:128, ib, 1, :],
                                              in_=qp[64:128, :])

                # --- K^T, staged to DRAM ---
                Wkv_r = Wkv_d.rearrange("(a p) i -> a p i", p=128)
                with tc.tile_pool(name="pa_wk", bufs=1) as pa_wk, \
                     tc.tile_pool(name="pa_stage", bufs=4) as pstg, \
                     tc.tile_pool(name="pa_ps2", bufs=4, space="PSUM") as ps2:
                    Wk_sb = pa_wk.tile([128, A, INNER], f32r)
                    for a in range(A):
                        nc.sync.dma_start(out=Wk_sb[:, a, :],
                                          in_=Wkv_r[a, :, 0:INNER])

                    for ib in range(IB):
                        for kc in range(KC):
                            kp = ps2.tile([128, 512], f32, tag="proj")
                            for a in range(A):
                                nc.tensor.matmul(
                                    out=kp,
                                    lhsT=Wk_sb[:, a, ib * 128:(ib + 1) * 128],
                                    rhs=xT_sb[:, a, kc * 512:(kc + 1) * 512],
                                    start=(a == 0), stop=(a == A - 1))
                            kstg = pstg.tile([128, 512], f32r, tag="stage")
                            nc.vector.tensor_copy(out=kstg, in_=kp)
                            nc.sync.dma_start(
                                out=KT_dram[ib * 128:(ib + 1) * 128,
                                            kc * 512:(kc + 1) * 512],
                                in_=kstg)

                # --- V, staged to DRAM ---
                with tc.tile_pool(name="pa_wv", bufs=1) as pa_wv, \
                     tc.tile_pool(name="pa_stage2", bufs=4) as pstg2, \
                     tc.tile_pool(name="pa_ps3", bufs=4, space="PSUM") as ps3:
                    Wv_sb = pa_wv.tile([128, A, INNER], f32r)
                    for a in range(A):
                        nc.sync.dma_start(out=Wv_sb[:, a, :],
                                          in_=Wkv_r[a, :, INNER:2 * INNER])

                    # ic-major so V columns for the first head pairs land in
                    # DRAM as early as possible (phase B consumes per-head
                    # column slices across all key blocks).
                    for ic in range(2):
                        for kb in range(KB):
                            vp = ps3.tile([128, 512], f32, tag="proj")
                            for a in range(A):
                                nc.tensor.matmul(
                                    out=vp,
                                    lhsT=xT_sb[:, a, kb * 128:(kb + 1) * 128],
                                    rhs=Wv_sb[:, a, ic * 512:(ic + 1) * 512],
                                    start=(a == 0), stop=(a == A - 1))
                            vstg = pstg2.tile([128, 512], f32r, tag="stage")
                            nc.vector.tensor_copy(out=vstg, in_=vp)
                            nc.sync.dma_start(
                                out=V_dram[kb * 128:(kb + 1) * 128,
                                           ic * 512:(ic + 1) * 512],
                                in_=vstg)

            # ---------------- Phases B + C ----------------
            with tc.tile_pool(name="pb_wo", bufs=1) as pwo:
                Wo_sb = pwo.tile([128, IB, DM], f32r)
                Wo_r = Wo_d.rearrange("(ib p) d -> ib p d", p=128)
                for ib in range(IB):
                    nc.sync.dma_start(out=Wo_sb[:, ib, :], in_=Wo_r[ib])

                V_r = V_dram.rearrange("(kb p) i -> p kb i", p=128)

                # --- Phase B: per-head attention ---
                with tc.tile_pool(name="pb_kt", bufs=2) as pkt, \
                     tc.tile_pool(name="pb_v", bufs=2) as pv, \
                     tc.tile_pool(name="pb_es", bufs=2) as pes, \
                     tc.tile_pool(name="pb_ps", bufs=1, space="PSUM") as psb:
                    for hp in range(H // 2):
                        KT_pair = pkt.tile([128, N], f32r, tag="kt")
                        nc.sync.dma_start(
                            out=KT_pair,
                            in_=KT_dram[hp * 128:(hp + 1) * 128, :])
                        for hh in range(2):
                            h = hp * 2 + hh
                            # [V_h | 1 | 0pad] -> full-width (M=128) lhsT
                            V_aug = pv.tile([128, KB, 128], f32r, tag="vaug")
                            nc.sync.dma_start(
                                out=V_aug[:, :, 0:64],
                                in_=V_r[:, :, h * 64:(h + 1) * 64])
                            nc.vector.tensor_copy(
                                out=V_aug[:, :, 64:65],
                                in_=onef.unsqueeze(1).to_broadcast([128, KB, 1]))
                            nc.vector.tensor_copy(
                                out=V_aug[:, :, 65:128],
                                in_=zerof.unsqueeze(1).to_broadcast(
                                    [128, KB, 63]))

                            expS = pes.tile([128, KB, QR], f32r, tag="es")
                            op = psb.tile([128, QR], f32, tag="o", bufs=2)
                            for kb in range(KB):
                                sp = psb.tile([128, QR], f32, tag="s", bufs=3)
                                nc.tensor.matmul(
                                    out=sp,
                                    lhsT=KT_pair[:, kb * 128:(kb + 1) * 128],
                                    rhs=QT_z[:, hp, hh, :],
                                    start=True, stop=True)
                                nc.scalar.activation(
                                    out=expS[:, kb, :], in_=sp, func=Exp,
                                    scale=SCALE)
                                nc.tensor.matmul(
                                    out=op,
                                    lhsT=V_aug[:, kb, :],
                                    rhs=expS[:, kb, :],
                                    start=(kb == 0), stop=(kb == KB - 1))
                            # 1/rowsum, broadcast across 64 partitions via a
                            # DRAM bounce (SBUF DMA reads can't be
                            # partition-stride-0; DRAM reads can).  Keeps the
                            # PE stream free of normalization dependencies.
                            recip = pv.tile([1, QR], f32, tag="recip")
                            nc.vector.reciprocal(out=recip, in_=op[64:65, :])
                            rcd = dram2.tile([1, QR], f32, tag="rcd")
                            nc.sync.dma_start(out=rcd, in_=recip)
                            rbs = pv.tile([64, QR], f32, tag="rbs")
                            nc.sync.dma_start(
                                out=rbs, in_=rcd.to_broadcast([64, QR]))
                            nc.vector.tensor_mul(
                                OT_sb[hh * 64:(hh + 1) * 64, hp, :],
                                op[0:64, :], rbs)

                # --- Phase C: output projection ---
                with tc.tile_pool(name="pc_out", bufs=4) as pout, \
                     tc.tile_pool(name="pc_ps", bufs=4, space="PSUM") as psc:
                    for qb in range(QB):
                        for dc in range(2):
                            outp = psc.tile([128, 512], f32, tag="out")
                            for ib in range(IB):
                                nc.tensor.matmul(
                                    out=outp,
                                    lhsT=OT_sb[:, ib, qb * 128:(qb + 1) * 128],
                                    rhs=Wo_sb[:, ib, dc * 512:(dc + 1) * 512],
                                    start=(ib == 0), stop=(ib == IB - 1))
                            ob = pout.tile([128, 512], f32, tag="ob")
                            nc.vector.tensor_add(
                                ob, outp, bo_sb[:, dc * 512:(dc + 1) * 512])
                            nc.sync.dma_start(
                                out=out_d[qb * 128:(qb + 1) * 128,
                                          dc * 512:(dc + 1) * 512],
                                in_=ob)

    nc.compile()
    return nc


MM_DTYPE = "f32r"


def _get_nc():
    if "nc" not in _cached:
        _cached["nc"] = _build(MM_DTYPE)
    return _cached["nc"]


def kernel(queries, Wq, Wkv, Wo, bo, _trace=False):
    from concourse.bass_utils import run_bass_kernel_spmd

    queries = np.asarray(queries, dtype=np.float32)
    Wq = np.asarray(Wq, dtype=np.float32)
    Wkv = np.asarray(Wkv, dtype=np.float32)
    Wo = np.asarray(Wo, dtype=np.float32)
    bo = np.asarray(bo, dtype=np.float32)

    nc = _get_nc()

    if MM_DTYPE == "bf16":
        import ml_dtypes
        cast = lambda a: a.astype(ml_dtypes.bfloat16)
    else:
        cast = lambda a: a
    Wq_c, Wkv_c, Wo_c = cast(Wq), cast(Wkv), cast(Wo)

    in_maps = []
    for c in range(NCORES):
        g, r = c // 4, c % 4
        xT = np.ascontiguousarray(queries[g].T)          # [DM, N]
        xT = cast(np.ascontiguousarray(np.roll(xT, -r * QR, axis=1)))
        in_maps.append({"xT": xT, "Wq": Wq_c, "Wkv": Wkv_c, "Wo": Wo_c,
                        "bo": bo})

    res = run_bass_kernel_spmd(nc, in_maps, list(range(NCORES)),
                               trace=_trace)
    out = np.empty((B, N, DM), dtype=np.float32)
    for c in range(NCORES):
        g, r = c // 4, c % 4
        out[g, r * QR:(r + 1) * QR, :] = res.results[c]["out"]
    if _trace:
        return out, res
    return out


if __name__ == "__main__":
    rng = np.random.default_rng(0)
    q = rng.standard_normal((B, N, DM), dtype=np.float32)
    s = 0.02
    inputs = dict(
        queries=q,
        Wq=(rng.standard_normal((DM, INNER), dtype=np.float32) * s),
        Wkv=(rng.standard_normal((DM, 2 * INNER), dtype=np.float32) * s),
        Wo=(rng.standard_normal((INNER, DM), dtype=np.float32) * s),
        bo=(rng.standard_normal((DM,), dtype=np.float32) * s),
    )
    out = kernel(**inputs)
    print("kernel ran, out shape", out.shape)
